# revision 73
# baseline (speedup 1.0000x reference)
"""Trainium2 Bass kernel: 3D bilateral filter (5x5x5, replicate pad).

Reference math (normalization of wd cancels in the final ratio):
    out(v) = sum_k g_k * exp(-a*(p_k - c)^2) * p_k / sum_k g_k * exp(-a*(p_k - c)^2)
with a = 1/(2*0.8^2), g the separable 5x5x5 gaussian, p_k the 125
replicate-padded shifted neighbours and c the center voxel.

Device strategy (per core, data-parallel over the 144 (c,d) planes, 18 each):
  - taps-on-partitions layout: im2col DMA materializes P[125, V] per block
    (V = 768 voxels = 16 rows x 48 cols of one output plane)
  - PE broadcasts the center row (tap 62) to 125 partitions via a ones[1,125]
    matmul into PSUM
  - DVE: D = P - C ; U = D*D ; T = E*P
  - ACT: E = Exp(-a*U + ln(g_k)) -- ln(g) rides the per-partition bias
  - PE: ones[125,1] matmul reduces [T | E] over taps into row b of a
    persistent PSUM accumulator (num | den per block)
  - epilogue: out = num * reciprocal(den), one DMA back to DRAM

All replicate padding AND the im2col layout are done host-side:
inh[o, k, :] is the 48x48 valid-region view of output plane o shifted by tap
k = (i, j, l) (replicate-padded), flattened to 2304 contiguous floats, so each
device block load is a single clean 2-dim DMA of [125, 768].
"""

import math
from contextlib import ExitStack

import numpy as np

import concourse.bass as bass
import concourse.mybir as mybir
import concourse.tile as tile
from concourse import bacc, dve_ops
from concourse.bass_utils import run_bass_kernel_spmd
from concourse.dve_spec import Spec, Src0, Src1, _has_src1, lower, sq
from concourse.dve_uop import DveOpSpec

F32 = mybir.dt.float32
F32R = mybir.dt.float32r
BF16 = mybir.dt.bfloat16


def _register_diffsq() -> "dve_ops.DveOp":
    """Register a fused (a-b)^2 custom DVE op (one pass instead of sub+mul)."""
    name = "DIFF_SQ_ANT"
    for op in dve_ops.OPS:
        if op.name == name:
            return op
    spec = Spec(
        body=sq(Src0 - Src1),
        reference=lambda in0, in1, s0, s1, imm2: (
            (in0.astype(np.float32) - in1.astype(np.float32)) ** 2
        ).astype(np.float32),
    )
    row = dve_ops._CUSTOM_DVE_ROW_BASE + len(dve_ops.OPS)
    assert row < 0x20
    shas = {}
    for ver in ("v3", "v4"):
        shas[ver] = DveOpSpec(
            name=name, opcode=row, uops=lower(spec, ver=ver), rd1_en=_has_src1(spec)
        ).sha(ver)
    op = dve_ops.DveOp(name, spec, subdim=False, uops_sha=shas)
    dve_ops.OPS.append(op)
    dve_ops.CUSTOM_DVE_SPECS[name] = spec
    dve_ops._SUB_OPCODE_FOR_NAME[name] = row
    return op


DIFF_SQ = _register_diffsq()

SIGMA = 0.8
NEG_A = -1.0 / (2.0 * SIGMA * SIGMA)  # -0.78125
KS = 5
NTAP = KS * KS * KS  # 125
NCORES = 8
C_, D_, H_, W_ = 3, 48, 48, 48
PPC = (C_ * D_) // NCORES  # 18 planes per core
HP = H_ + 4  # 52 padded rows/cols
RPB = 16  # output rows per block
NBI = H_ // RPB  # 3 blocks per plane
V = RPB * W_  # 768 free elements per block
NBLK = PPC * NBI  # 54 blocks per core
PLANE_V = H_ * W_  # 2304 valid voxels per plane
# taps reordered host-side so the center tap (2,2,2) sits on partition 0:
# PE matmul rhs must start at partition 0/32/64
TAP_PERM = [62] + [k for k in range(NTAP) if k != 62]


def _gauss_ln() -> np.ndarray:
    """ln of the normalized separable gaussian, float32 [125]."""
    sig = [0.3 * ((k - 1) * 0.5 - 1.0) + 0.8 for k in (KS, KS, KS)]
    grids = np.meshgrid(*[np.arange(k) for k in (KS, KS, KS)], indexing="ij")
    ker = np.ones((KS, KS, KS), dtype=np.float64)
    for k, s, m in zip((KS, KS, KS), sig, grids):
        mean = (k - 1) / 2.0
        ker = ker * np.exp(-((m - mean) ** 2) / (2.0 * s * s))
    ker = ker / ker.sum()
    return np.log(ker).astype(np.float32).reshape(-1)


def _kernel_body(
    ctx: ExitStack,
    tc: "tile.TileContext",
    inh,
    cen,
    lng,
    ones,
    zo,
    outp,
    repeat: int = 1,
):
    nc = tc.nc

    consts = ctx.enter_context(tc.tile_pool(name="consts", bufs=1))
    p_pool = ctx.enter_context(tc.tile_pool(name="p", bufs=8))
    u_pool = ctx.enter_context(tc.tile_pool(name="u", bufs=4))
    rhs_pool = ctx.enter_context(tc.tile_pool(name="rhs", bufs=4))
    epi_pool = ctx.enter_context(tc.tile_pool(name="epi", bufs=1))
    psc_pool = ctx.enter_context(tc.tile_pool(name="psc", bufs=2, space="PSUM"))
    acc_pool = ctx.enter_context(tc.tile_pool(name="acc", bufs=1, space="PSUM"))

    # constants (matmul weights DMA'd as float32r so the BIR verifier sees
    # fp32r-typed producers for every fp32r matmul operand)
    lng_t = consts.tile([NTAP, 1], F32)
    nc.sync.dma_start(lng_t[:], lng[:])
    ones_row = consts.tile([1, NTAP], F32R)
    nc.sync.dma_start(ones_row[:], ones[:].bitcast(F32R))
    # zo[:, 64] == 1 else 0: sliding window zo[:, 64-b:128-b] is the one-hot
    # lhsT that routes block b's tap-reduction into PSUM row b
    zo_t = consts.tile([NTAP, 128], F32R)
    nc.sync.dma_start(zo_t[:], zo[:].bitcast(F32R))

    # persistent accumulator: row b = [num | pad | den | pad] of block b.
    # num and den each own two whole PSUM banks so every bank has exactly one
    # start=True accumulation chain (groups are tracked per bank).
    acc = acc_pool.tile([128, 2048], F32)

    # PE matmuls only support a single sync-wait: consume the const-DMA
    # semaphores with throwaway matmuls so real ones wait on one producer only
    nc.tensor.matmul(
        acc[0:NTAP, 0:1], ones_row[:].bitcast(F32), ones_row[:, 0:1].bitcast(F32),
        start=True, stop=True, skip_group_check=True,
    )
    nc.tensor.matmul(
        acc[0:1, 0:1], zo_t[:, 0:1].bitcast(F32), zo_t[:, 0:1].bitcast(F32),
        start=True, stop=True, skip_group_check=True,
    )

    for _rep in range(repeat):
      for o in range(PPC):
        for bi in range(NBI):
            b = o * NBI + bi
            n0, n1 = bi * V, (bi + 1) * V
            # --- im2col load: P[k, v] = inh[o, perm[k], bi*768 + v]
            p_t = p_pool.tile([NTAP, V], F32)
            nc.sync.dma_start(p_t[:], inh[o, :, n0:n1])

            # center row (partition 0; taps are center-first-permuted), f32r
            c_row = p_pool.tile([1, V], F32R, tag="crow")
            nc.sync.dma_start(c_row[:], cen[o, n0:n1].bitcast(F32R))

            # --- broadcast center row to 125 partitions (PSUM), float32r
            c_t = psc_pool.tile([NTAP, V], F32)
            for m0 in range(0, V, 512):
                m1 = min(m0 + 512, V)
                nc.tensor.matmul(
                    c_t[:, m0:m1],
                    ones_row[:],
                    c_row[:, m0:m1],
                    start=True,
                    stop=True,
                )

            # --- U = (P - C)^2, one fused DVE pass
            u_t = u_pool.tile([NTAP, V], F32)
            nc.vector._custom_dve(DIFF_SQ, out=u_t[:], in0=p_t[:], in1=c_t[:])

            # --- E = exp(-a*U + ln g), rounded to f32r for the PE reduce
            e_t = rhs_pool.tile([NTAP, V], F32R, tag="e")
            nc.scalar.activation(
                e_t[:],
                u_t[:],
                mybir.ActivationFunctionType.Exp,
                bias=lng_t[:],
                scale=NEG_A,
            )
            # --- T = E * P (keep on DVE: a Pool-offload experiment measured
            # 258us vs 164us — real-HW Pool tensor ops are far slower than
            # the cost model's 0.42 efficiency suggests)
            t_t = rhs_pool.tile([NTAP, V], F32R, tag="t")
            nc.vector.tensor_mul(t_t[:], e_t[:], p_t[:])

            # --- reduce taps: num of block b -> acc row b, den -> acc row 64+b
            # each matmul reads one producer's tile and stays inside one PSUM
            # bank (bank boundaries at 512-float multiples of acc)
            chunks = (
                (t_t, 0, 0, 512),
                (t_t, 0, 512, V),
                (e_t, 1024, 0, 512),
                (e_t, 1024, 512, V),
            )
            for src_t, h0, c0, c1 in chunks:
                nc.tensor.matmul(
                    acc[0:64, h0 + c0 : h0 + c1],
                    zo_t[:, 64 - b : 128 - b],
                    src_t[:, c0:c1],
                    start=(b == 0),
                    stop=(b == NBLK - 1),
                )

    # --- epilogue: out = num / den
    recip_t = epi_pool.tile([NBLK, V], F32)
    nc.vector.reciprocal(recip_t[:], acc[0:NBLK, 1024 : 1024 + V])
    out_t = epi_pool.tile([NBLK, V], F32)
    nc.vector.tensor_mul(out_t[:], acc[0:NBLK, 0:V], recip_t[:])

    dst = outp.rearrange("o (b r) w -> (o b) (r w)", b=NBI)
    nc.sync.dma_start(dst, out_t[:])


def build_program(repeat: int = 1) -> bass.Bass:
    nc = bacc.Bacc("TRN2", target_bir_lowering=False, debug=False)
    inh = nc.declare_dram_parameter("inh", [PPC, NTAP, PLANE_V], F32, isOutput=False)
    cen = nc.declare_dram_parameter("cen", [PPC, PLANE_V], F32, isOutput=False)
    lng = nc.declare_dram_parameter("lng", [NTAP], F32, isOutput=False)
    ones = nc.declare_dram_parameter("ones", [NTAP], F32, isOutput=False)
    zo = nc.declare_dram_parameter("zo", [NTAP, 128], F32, isOutput=False)
    outp = nc.declare_dram_parameter("out", [PPC, H_, W_], F32, isOutput=True)
    with tile.TileContext(nc) as tc, ExitStack() as ctx:
        _kernel_body(ctx, tc, inh, cen, lng, ones, zo, outp, repeat=repeat)
    nc.compile()
    return nc


def build_host_inputs(x: np.ndarray) -> list[dict[str, np.ndarray]]:
    """x: [1, 3, 48, 48, 48] float32 -> per-core in_maps."""
    x = np.asarray(x).reshape(C_, D_, H_, W_).astype(np.float32)
    xp = np.pad(x, ((0, 0), (0, 0), (2, 2), (2, 2)), mode="edge")  # [3,48,52,52]
    lng = _gauss_ln()[TAP_PERM].copy()
    ones = np.ones(NTAP, dtype=np.float32)
    zo = np.zeros((NTAP, 128), dtype=np.float32)
    zo[:, 64] = 1.0
    in_maps = []
    for m in range(NCORES):
        inh = np.empty((PPC, NTAP, PLANE_V), dtype=np.float32)
        for o in range(PPC):
            q = m * PPC + o
            c, d = divmod(q, D_)
            for i in range(KS):
                dd = min(max(d + i - 2, 0), D_ - 1)
                win = np.lib.stride_tricks.sliding_window_view(
                    xp[c, dd], (H_, W_)
                )  # [5, 5, 48, 48]
                inh[o, i * 25 : (i + 1) * 25] = win.reshape(25, PLANE_V)
            inh[o] = inh[o, TAP_PERM]
        cenm = np.ascontiguousarray(inh[:, 0, :])
        in_maps.append(
            {"inh": inh, "cen": cenm, "lng": lng, "ones": ones, "zo": zo}
        )
    return in_maps


_PROGRAM: bass.Bass | None = None


def _get_program() -> bass.Bass:
    global _PROGRAM
    if _PROGRAM is None:
        _PROGRAM = build_program()
    return _PROGRAM


def kernel(x: np.ndarray) -> np.ndarray:
    nc = _get_program()
    in_maps = build_host_inputs(x)
    res = run_bass_kernel_spmd(nc, in_maps, list(range(NCORES)))
    planes = np.concatenate(
        [res.results[m]["out"].reshape(PPC, H_, W_) for m in range(NCORES)], axis=0
    )  # [144, 48, 48]
    return planes.reshape(1, C_, D_, H_, W_).astype(np.float32)
